# revision 84
# baseline (speedup 1.0000x reference)
"""Trainium2 Bass kernel v4 for nn_MiddleLayerEncoder (gnn_message_passing).

Strategy: shard by CLUSTER across 8 cores (512 whole clusters each, no
collectives).  Host prep sorts points by cluster and packs whole clusters
into 1024-column bins with a UNIFORM padded width per bin (canonical
across cores -> single SPMD program).  Uniform widths make every segment
reduce a single strided DVE op per bin.

Per-bin L1 lhsT slots: rows 0:67 = static W1ab (preloaded once), rows
96:96+nk = per-bin M (the per-cluster term M = [neigh;1]^T @ [W1c;b1],
evacuated from PSUM by an Act copy; 96 keeps the copy quadrant-aligned,
rows 67:96 are zeroed once so stale encT rows multiply to zero).  encT
carries payload rows 0:67 (pts 3 + feat 64) and nk bin-local one-hot
rows at 96, so the L1 matmul injects M per point for free in K.

Segment maxes are one DVE reduce per bin directly from PSUM (stage1: 4D
[p,n,h,wq] fused over the two enc2 halves; stage2: 3D [p,n,W]); the tiny
partition fold rides the idle gpsimd iDMA with a full iteration of slack
(LA=3 enc lookahead) so the in-order DVE queue never waits on it.  PSUM:
psE(1x2banks) enc, psL(3x2banks) ring = L1 b0/b1, next st's two M
matmuls, L2 b0/b1.  The encT stream is split into three SP-ring DMAs per
supertile -- empirically the sweet spot for the greedy descriptor
dispatcher that otherwise piles everything onto one DMA queue.  Output
is written bf16 (widened on host).
"""

import numpy as np
import ml_dtypes
from contextlib import ExitStack

import concourse.bass as bass
import concourse.bacc as bacc
import concourse.tile as tile
from concourse import mybir
from concourse.bass_utils import run_bass_kernel_spmd

BF16 = mybir.dt.bfloat16
F32 = mybir.dt.float32
NPBF16 = ml_dtypes.bfloat16

N_CORES = 8
N_PTS = 262144
TAIL_SPLIT = False
N_CLUSTERS = 4096
BIN = 1024
ST = 2 * BIN
NK_MAX = 30        # one-hot rows per bin: 67 + nk <= 128 (and <= 97 here)

# bf16 weight blob layout: (name, K rows, cols, partition base)
_WB16 = [
    ("enc1_lhsT", 12, 128, 0), ("enc2_lhsT", 64, 128, 0),
    ("enc2_hi", 64, 128, 64), ("W1ab", 67, 128, 0), ("W1cb1", 65, 128, 0),
    ("fcW2", 128, 128, 0), ("G1", 128, 128, 0),
    ("G2a", 128, 128, 0), ("G2b", 128, 128, 0),
]
_WF32 = ["b_enc1_4", "b_enc2", "b2", "gb1", "gb2a", "gb2b"]  # one f32 col each


# ---------------------------------------------------------------- planning

def _plan(cluster):
    """Canonical SPMD layout shared by all cores (uniform width per bin)."""
    counts = np.bincount(cluster, minlength=N_CLUSTERS)
    assert counts.min() >= 1, "empty cluster unsupported"
    order = np.argsort(-counts, kind="stable")  # cluster ids, size desc

    n_ranks = N_CLUSTERS // N_CORES
    cids = np.empty((N_CORES, n_ranks), dtype=np.int64)
    for i, cid in enumerate(order):
        rnd, pos = divmod(i, N_CORES)
        core = pos if rnd % 2 == 0 else N_CORES - 1 - pos
        cids[core, rnd] = cid

    sizes = counts[cids]                      # [cores, ranks]
    Lmax = sizes.max(axis=0)                  # canonical per-rank size, desc

    # uniform-width bins: consecutive ranks (size desc) packed into 1024-col
    # bins; every cluster in a bin is padded to W = pad4(first rank's size)
    bins = []          # (r0, n, W)
    r0 = 0
    while r0 < n_ranks:
        W = int((Lmax[r0] + 3) // 4 * 4)
        n = min(BIN // W, n_ranks - r0, NK_MAX)
        bins.append((r0, int(n), W))
        r0 += n
    if len(bins) % 2:
        bins.append((n_ranks, 0, 0))          # empty bin pads to whole st

    L = np.zeros(n_ranks, dtype=np.int64)
    col0 = np.zeros(n_ranks, dtype=np.int64)
    for b, (r0, n, W) in enumerate(bins):
        for j in range(n):
            L[r0 + j] = W
            col0[r0 + j] = BIN * b + j * W
    S = BIN * len(bins)
    nk_rows = 96 + max(n for (_, n, _) in bins)

    # distinct one-hot geometries (n, W): a master pattern per geometry
    # lives in SBUF; per-bin it is copied on-chip instead of DMA'd from HBM
    geoms = []
    geom_of = []
    for (r0, n, W) in bins:
        key = (n, W)
        if n > 0 and key not in geoms:
            geoms.append(key)
        geom_of.append(geoms.index(key) if n > 0 else -1)

    sts = [(bins[b], bins[b + 1]) for b in range(0, len(bins), 2)]
    return dict(cids=cids, L=L, col0=col0, S=S, bins=bins, sts=sts,
                n_ranks=n_ranks, nk_rows=nk_rows, geoms=geoms,
                geom_of=geom_of)


def _prep_core(k, plan, rel_points, features, sort_idx, bucket0):
    """Per-core input arrays (canonical layout, core-specific data)."""
    col0, S, L = plan["col0"], plan["S"], plan["L"]
    cids = plan["cids"][k]
    n_ranks = plan["n_ranks"]
    nk_rows = plan["nk_rows"]

    # gap columns (bin tails) keep index 0; they are never reduced
    slot = np.zeros(S, dtype=np.int64)
    for r in range(n_ranks):
        cid = cids[r]
        idx = sort_idx[bucket0[cid]: bucket0[cid + 1]]
        n = idx.shape[0]
        c0 = col0[r]
        slot[c0: c0 + n] = idx
        if L[r] > n:
            slot[c0 + n: c0 + L[r]] = idx[0]

    pts = rel_points[slot]          # [S, 3] f32
    feat = features[slot]           # [S, 64] f32

    # encT rows: 0:3 points, 3:67 features, 96:96+nk bin-local one-hot
    encT = np.zeros((nk_rows, S), dtype=NPBF16)
    encT[0:3] = pts.T.astype(NPBF16)
    encT[3:67] = feat.T.astype(NPBF16)
    for b, (r0, n, W) in enumerate(plan["bins"]):
        for j in range(n):
            c0 = BIN * b + j * W
            encT[96 + j, c0:c0 + W] = NPBF16(1.0)

    pts4 = (
        pts.astype(NPBF16)
        .reshape(S // 4, 4, 3)
        .transpose(1, 2, 0)
        .reshape(12, S // 4)
    )
    return {"encT": encT, "pts4": np.ascontiguousarray(pts4)}


def _blockdiag(w, times):
    fi, fo = w.shape
    out = np.zeros((fi * times, fo * times), dtype=w.dtype)
    for i in range(times):
        out[i * fi:(i + 1) * fi, i * fo:(i + 1) * fo] = w
    return out


def _prep_weights(inp):
    W1 = inp["W1"]
    mats = {
        "enc1_lhsT": _blockdiag(inp["enc_W1"], 4),
        "enc2_lhsT": _blockdiag(inp["enc_W2"], 2),
        "enc2_hi": _blockdiag(inp["enc_W2"], 2),
        "W1ab": W1[0:67],
        "W1cb1": np.vstack([W1[67:131], inp["b1"][None]]),
        "fcW2": inp["W2"], "G1": inp["G1"],
        "G2a": inp["G2"][:, 0:128], "G2b": inp["G2"][:, 128:256],
    }
    wb16 = np.zeros((128, 128 * len(_WB16)), dtype=NPBF16)
    for i, (name, k, cols, pbase) in enumerate(_WB16):
        wb16[pbase:pbase + k, 128 * i:128 * i + cols] = mats[name].astype(NPBF16)

    vecs = {
        "b_enc1_4": np.tile(inp["enc_b1"], 4), "b_enc2": inp["enc_b2"],
        "b2": inp["b2"], "gb1": inp["gb1"],
        "gb2a": inp["gb2"][0:128], "gb2b": inp["gb2"][128:256],
    }
    wf32 = np.zeros((128, len(_WF32)), dtype=np.float32)
    for i, name in enumerate(_WF32):
        v = vecs[name]
        wf32[0:v.shape[0], i] = v
    return {"wb16": wb16, "wf32": wf32}


# ---------------------------------------------------------------- program

def _build(plan):
    S = plan["S"]
    n_ranks = plan["n_ranks"]
    nk_rows = plan["nk_rows"]
    nc = bacc.Bacc(None, target_bir_lowering=False, debug=True)

    encT_d = nc.dram_tensor("encT", [nk_rows, S], BF16, kind="ExternalInput")
    pts4_d = nc.dram_tensor("pts4", [12, S // 4], BF16, kind="ExternalInput")
    wb16_d = nc.dram_tensor("wb16", [128, 128 * len(_WB16)], BF16,
                            kind="ExternalInput")
    wf32_d = nc.dram_tensor("wf32", [128, len(_WF32)], F32, kind="ExternalInput")
    out_d = nc.dram_tensor("out", [256, 512], BF16, kind="ExternalOutput")

    RELU = mybir.ActivationFunctionType.Relu
    ADD = mybir.AluOpType.add
    MAX = mybir.AluOpType.max
    AX = mybir.AxisListType.X
    AXY = mybir.AxisListType.XY

    sts = plan["sts"]
    n_st = len(sts)
    W1AB_COL = 128 * 3  # W1ab offset in the bf16 blob
    N_SLOT = 4

    with tile.TileContext(nc) as tc, ExitStack() as ctx:
        consts = ctx.enter_context(tc.tile_pool(name="consts", bufs=1))
        glob = ctx.enter_context(tc.tile_pool(name="glob", bufs=1))
        enc_p = ctx.enter_context(tc.tile_pool(name="enc_p", bufs=4))
        h1_p = ctx.enter_context(tc.tile_pool(name="h1_p", bufs=2))
        e1_p = ctx.enter_context(tc.tile_pool(name="e1_p", bufs=2))
        sm_p = ctx.enter_context(tc.tile_pool(name="sm_p", bufs=3))
        psE = ctx.enter_context(tc.tile_pool(name="psE", bufs=1, space="PSUM"))
        psL = ctx.enter_context(tc.tile_pool(name="psL", bufs=3, space="PSUM"))

        # startup order: enc-stage weights + points + biases first (enough
        # for the prologue's enc matmuls), then the bulk of the blob
        wb16_t = consts.tile([128, 128 * len(_WB16)], BF16, tag="wb16")
        wf32_t = consts.tile([128, len(_WF32)], F32, tag="wf32")
        nc.sync.dma_start(out=wb16_t[:, 0:384], in_=wb16_d[:, 0:384])

        w_sb = {}
        for i, (name, k, cols, pbase) in enumerate(_WB16):
            w_sb[name] = wb16_t[pbase:pbase + k, 128 * i:128 * i + cols]
        bias = {}
        for i, name in enumerate(_WF32):
            rows = {"b_enc2": 64}.get(name, 128)
            bias[name] = wf32_t[0:rows, i:i + 1]

        # resident quad-packed points
        pts4_sb = consts.tile([12, S // 4], BF16, tag="pts4_sb")
        nc.sync.dma_start(out=pts4_sb[:], in_=pts4_d[:])
        nc.sync.dma_start(out=wf32_t[:], in_=wf32_d[:])
        nc.sync.dma_start(out=wb16_t[:, 384:], in_=wb16_d[:, 384:])

        # per-bin L1 lhsT slots: rows 0:67 = static W1ab, rows 96:96+nk = M,
        # rows 67:96 zero (paired with stale encT rows)
        lhsT_slots = [consts.tile([128, 128], BF16, tag=f"lhsT{i}",
                                  name=f"lhsT{i}") for i in range(N_SLOT)]
        for sl in lhsT_slots:
            nc.vector.memset(sl[64:96, :], 0.0)
            nc.scalar.dma_start(out=sl[0:67, :],
                                in_=wb16_d[0:67, W1AB_COL:W1AB_COL + 128])

        # neighT: [65, n_ranks]; row 64 = ones (folds b1 into the M matmul)
        neighT = glob.tile([65, n_ranks], BF16, tag="neighT")
        nc.vector.memset(neighT[64:65, :], 1.0)
        # stage-2 partials for the whole core
        T2 = glob.tile([128, n_ranks], BF16, tag="T2")

        # --------------- pipeline stages (per supertile) ---------------

        def enc_head(si):
            """enc1 matmul + relu evac; prefetch encT; returns psum+tiles."""
            (b0, b1) = sts[si]
            c0 = si * ST
            q0 = c0 // 4   # 512 quads per supertile
            ab = psE.tile([128, BIN], F32, tag="psE", name="ab")
            nc.tensor.matmul(ab[:, 0:512], w_sb["enc1_lhsT"],
                             pts4_sb[:, q0:q0 + 512], start=True, stop=True)
            h1 = h1_p.tile([128, 512], BF16, tag="h1")
            nc.scalar.activation(h1[:], ab[:, 0:512], RELU,
                                 bias=bias["b_enc1_4"], scale=1.0)
            # prefetch this supertile's encT (payload rows via SP DGE, the
            # small one-hot rows via the idle gpsimd DGE)
            encT_t = enc_p.tile([nk_rows, ST], BF16, tag="encT_t")
            if si < 4:
                # one-time per pool buffer: rows 67:96 stay zero forever
                # (uninitialized SBUF could hold NaN; 0 * NaN = NaN)
                nc.vector.memset(encT_t[64:96, :], 0.0)
            nc.sync.dma_start(out=encT_t[0:34, :],
                              in_=encT_d[0:34, c0:c0 + ST])
            nc.sync.dma_start(out=encT_t[34:67, :],
                              in_=encT_d[34:67, c0:c0 + ST])
            rows = 96 + max(b0[1], b1[1])
            nc.sync.dma_start(out=encT_t[96:rows, :],
                              in_=encT_d[96:rows, c0:c0 + ST])
            return ab, h1, encT_t

        def enc_tail(si, ab, h1):
            """enc2 matmuls + per-bin stage-1 reduce -> neighT columns."""
            (b0, b1) = sts[si]
            nk = b0[1] + b1[1]
            nc.tensor.matmul(ab[:, 0:512], w_sb["enc2_lhsT"], h1[0:64, :],
                             start=True, stop=True)
            nc.tensor.matmul(ab[:, 512:1024], w_sb["enc2_hi"], h1[64:128, :],
                             start=True, stop=True)
            # per-bin fused reduce over (half h, quad cols), separately for
            # the low/high 64-partition point groups (aligned starts), then
            # a partition-free tensor_max folds them -- no shift DMA needed
            mx = sm_p.tile([128, NK_MAX * 2], BF16, tag="mx")
            halves = ab[:].rearrange("p (h c) -> p h c", h=2)
            o = 0
            for t, (r0, n, W) in enumerate((b0, b1)):
                if n == 0:
                    continue
                wq = W // 4
                q0 = t * (BIN // 4)
                nc.vector.reduce_max(
                    mx[:, o:o + n],
                    halves[:, :, q0:q0 + n * wq]
                    .rearrange("p h (n w) -> p n h w", w=wq),
                    axis=AXY)
                o += n
            # partition fold via the idle gpsimd iDMA (tiny copy); the
            # dependent mx2/bias ops are issued later (enc_tail_b) so the
            # in-order DVE queue never stalls waiting for this DMA
            fold = sm_p.tile([64, NK_MAX * 2], BF16, tag="fold")
            nc.gpsimd.dma_start(out=fold[:, :nk], in_=mx[64:128, :nk])
            return mx, fold

        def enc_tail_b(si, mx, fold):
            (b0, b1) = sts[si]
            nk = b0[1] + b1[1]
            mx2 = sm_p.tile([64, NK_MAX * 2], BF16, tag="mx2")
            nc.vector.tensor_max(mx2[:, :nk], mx[0:64, :nk], fold[:, :nk])
            r0 = b0[0]
            nc.scalar.activation(neighT[0:64, r0:r0 + nk], mx2[:, :nk],
                                 RELU, bias=bias["b_enc2"], scale=1.0)

        def m_chain(si):
            """Per-bin M = [neigh;1]^T @ [W1c;b1] -> lhsT slot rows 67:67+nk."""
            (b0, b1) = sts[si]
            for t, (r0, n, W) in enumerate((b0, b1)):
                if n == 0:
                    continue
                pm = psL.tile([128, BIN], F32, tag="psL", name="pm")
                nc.tensor.matmul(pm[:n, 0:128], neighT[0:65, r0:r0 + n],
                                 w_sb["W1cb1"], start=True, stop=True)
                sl = lhsT_slots[(2 * si + t) % N_SLOT]
                nc.scalar.copy(sl[96:96 + n, :], pm[:n, 0:128])

        def l1_part(si, encT_t):
            """Per-bin L1 matmuls; relu evacs trail behind."""
            (b0, b1) = sts[si]
            e1 = e1_p.tile([128, ST], BF16, tag="e1")
            p1s = []
            for t, (r0, n, W) in enumerate((b0, b1)):
                fill = n * W
                sl = lhsT_slots[(2 * si + t) % N_SLOT]
                p1 = psL.tile([128, BIN], F32, tag="psL", name="p1")
                for a in (0, 512):
                    z = min(512, fill - a)
                    if z > 0:
                        nc.tensor.matmul(
                            p1[:, a:a + z], sl[0:96 + n, :],
                            encT_t[0:96 + n, t * BIN + a:t * BIN + a + z],
                            start=True, stop=True)
                p1s.append((p1, fill))
            for t, (p1, fill) in enumerate(p1s):
                if fill > 0:
                    nc.scalar.activation(e1[:, t * BIN:t * BIN + fill],
                                         p1[:, :fill], RELU)
            return e1

        def l2_part(si, e1):
            (b0, b1) = sts[si]
            for t, (r0, n, W) in enumerate((b0, b1)):
                fill = n * W
                if fill == 0:
                    continue
                p2 = psL.tile([128, BIN], F32, tag="psL", name="p2")
                for a in (0, 512):
                    z = min(512, fill - a)
                    if z > 0:
                        nc.tensor.matmul(
                            p2[:, a:a + z], w_sb["fcW2"],
                            e1[:, t * BIN + a:t * BIN + a + z],
                            start=True, stop=True)
                nc.vector.reduce_max(
                    T2[:, r0:r0 + n],
                    p2[:, 0:fill].rearrange("p (n w) -> p n w", w=W),
                    axis=AX)

        # --------------- software pipeline, lookahead 2 ---------------
        # per-iteration order keeps every tensor matmul's producer one full
        # iteration ahead: L1(si) needs M(si) (copied in iter si-1), the m
        # matmuls need neighT(si+1) (stage1 of iter si-1), L2 needs this
        # iteration's early Act evacs.
        # ---------------- global MLP tail (per rank chunk) ----------------
        gT = glob.tile([128, n_ranks], BF16, tag="gT")

        def tail_chunk(ck, r0, r1):
            z = r1 - r0
            if z <= 0:
                return
            nc.vector.tensor_scalar(gT[:, r0:r1], T2[:, r0:r1], bias["b2"],
                                    0.0, op0=ADD, op1=MAX)
            pg = psL.tile([128, BIN], F32, tag="psL", name="pg")
            nc.tensor.matmul(pg[:, 0:z], w_sb["G1"], gT[:, r0:r1],
                             start=True, stop=True)
            g1T = glob.tile([128, 512], BF16, tag=f"g1T{ck}")
            nc.scalar.activation(g1T[:, 0:z], pg[:, 0:z], RELU,
                                 bias=bias["gb1"], scale=1.0)
            for half, (wn, bn) in enumerate((("G2a", "gb2a"), ("G2b", "gb2b"))):
                po = psL.tile([128, BIN], F32, tag="psL", name="po")
                nc.tensor.matmul(po[:, 0:z], w_sb[wn], g1T[:, 0:z],
                                 start=True, stop=True)
                o_sb = glob.tile([128, 512], BF16, tag=f"osb{ck}_{half}")
                nc.scalar.activation(o_sb[:, 0:z], po[:, 0:z], RELU,
                                     bias=bias[bn], scale=1.0)
                nc.scalar.dma_start(
                    out=out_d[half * 128:(half + 1) * 128, r0:r1],
                    in_=o_sb[:, 0:z])

        # first tail chunk fires once its ranks' stage-2 maxes are final
        ranks_done = [sts[k][1][0] + sts[k][1][1] if sts[k][1][1] > 0
                      else sts[k][0][0] + sts[k][0][1] for k in range(n_st)]
        si_split = next((k for k, rd in enumerate(ranks_done)
                         if rd >= n_ranks // 2), n_st - 1)
        split_rank = ranks_done[si_split]

        # enc runs 3 supertiles ahead so the tiny partition-fold DMA gets a
        # full iteration of slack before mx2/neighT consume it on the DVE
        LA = 3
        pend = {}    # si -> (ab, h1, encT_t)
        mxs = {}     # si -> (mx, fold)
        for si in range(min(LA, n_st)):
            pend[si] = enc_head(si)
            mxs[si] = enc_tail(si, pend[si][0], pend[si][1])
        for si in range(min(LA - 1, n_st)):
            enc_tail_b(si, *mxs.pop(si))
        if n_st > 0:
            m_chain(0)
        for si in range(n_st):
            if si + LA < n_st:
                pend[si + LA] = enc_head(si + LA)
            e1 = l1_part(si, pend[si][2])
            if si + LA < n_st:
                mxs[si + LA] = enc_tail(si + LA, pend[si + LA][0],
                                        pend[si + LA][1])
            if si + 1 < n_st:
                m_chain(si + 1)
            l2_part(si, e1)
            if TAIL_SPLIT and si == si_split:
                tail_chunk(0, 0, split_rank)
            if si + LA - 1 in mxs:
                enc_tail_b(si + LA - 1, *mxs.pop(si + LA - 1))
            pend.pop(si)

        if TAIL_SPLIT:
            tail_chunk(1, split_rank, n_ranks)
        else:
            tail_chunk(1, 0, n_ranks)

    nc.finalize()
    return nc


# ---------------------------------------------------------------- entry

_CACHE = {}


def _run(inputs, trace=False, **spmd_kwargs):
    cluster = np.asarray(inputs["cluster"])
    key = hash(cluster.tobytes())
    if key not in _CACHE:
        plan = _plan(cluster)
        nc = _build(plan)
        _CACHE[key] = (plan, nc)
    plan, nc = _CACHE[key]

    rel_points = np.asarray(inputs["relative_points"], dtype=np.float32)
    features = np.asarray(inputs["features"], dtype=np.float32)
    sort_idx = np.argsort(cluster, kind="stable")
    bucket0 = np.concatenate(
        [[0], np.cumsum(np.bincount(cluster, minlength=N_CLUSTERS))]
    )
    wmap = _prep_weights({k: np.asarray(v, dtype=np.float32)
                          for k, v in inputs.items()
                          if k not in ("relative_points", "features", "cluster")})

    in_maps = []
    for k in range(N_CORES):
        m = _prep_core(k, plan, rel_points, features, sort_idx, bucket0)
        m.update(wmap)
        in_maps.append(m)

    res = run_bass_kernel_spmd(nc, in_maps, list(range(N_CORES)),
                               trace=trace, **spmd_kwargs)

    out = np.empty((N_CLUSTERS, 256), dtype=np.float32)
    for k in range(N_CORES):
        out[plan["cids"][k]] = res.results[k]["out"].T.astype(np.float32)
    return out, res


def kernel(**inputs):
    return _run(inputs)[0]


# revision 86
# speedup vs baseline: 1.0411x; 1.0411x over previous
"""Trainium2 Bass kernel v4 for nn_MiddleLayerEncoder (gnn_message_passing).

Strategy: shard by CLUSTER across 8 cores (512 whole clusters each, no
collectives).  Host prep sorts points by cluster and packs whole clusters
into 1024-column bins with a UNIFORM padded width per bin (canonical
across cores -> single SPMD program).  Uniform widths make every segment
reduce a single strided DVE op per bin.

Per-bin L1 lhsT slots: rows 0:67 = static W1ab (preloaded once), rows
96:96+nk = per-bin M (the per-cluster term M = [neigh;1]^T @ [W1c;b1],
evacuated from PSUM by an Act copy; 96 keeps the copy quadrant-aligned,
rows 67:96 are zeroed once so stale encT rows multiply to zero).  encT
carries payload rows 0:67 (pts 3 + feat 64) and nk bin-local one-hot
rows at 96, so the L1 matmul injects M per point for free in K.

Segment maxes are one DVE reduce per bin directly from PSUM (stage1: 4D
[p,n,h,wq] fused over the two enc2 halves; stage2: 3D [p,n,W]); the tiny
partition fold rides the idle gpsimd iDMA with a full iteration of slack
(LA=3 enc lookahead) so the in-order DVE queue never waits on it.  PSUM:
psE(1x2banks) enc, psL(3x2banks) ring = L1 b0/b1, next st's two M
matmuls, L2 b0/b1.  The encT stream is split into three SP-ring DMAs per
supertile -- empirically the sweet spot for the greedy descriptor
dispatcher that otherwise piles everything onto one DMA queue.  Output
is written bf16 (widened on host).
"""

import numpy as np
import ml_dtypes
from contextlib import ExitStack

import concourse.bass as bass
import concourse.bacc as bacc
import concourse.tile as tile
from concourse import mybir
from concourse.bass_utils import run_bass_kernel_spmd

BF16 = mybir.dt.bfloat16
F32 = mybir.dt.float32
NPBF16 = ml_dtypes.bfloat16

N_CORES = 8
N_PTS = 262144
TAIL_SPLIT = False
N_CLUSTERS = 4096
BIN = 1024
ST = 2 * BIN
NK_MAX = 30        # one-hot rows per bin: 67 + nk <= 128 (and <= 97 here)

# bf16 weight blob layout: (name, K rows, cols, partition base)
_WB16 = [
    ("enc1_lhsT", 12, 128, 0), ("enc2_lhsT", 64, 128, 0),
    ("enc2_hi", 64, 128, 64), ("W1ab", 67, 128, 0), ("W1cb1", 65, 128, 0),
    ("fcW2", 128, 128, 0), ("G1", 128, 128, 0),
    ("G2a", 128, 128, 0), ("G2b", 128, 128, 0),
]
_WF32 = ["b_enc1_4", "b_enc2", "b2", "gb1", "gb2a", "gb2b"]  # one f32 col each


# ---------------------------------------------------------------- planning

def _plan(cluster):
    """Canonical SPMD layout shared by all cores (uniform width per bin)."""
    counts = np.bincount(cluster, minlength=N_CLUSTERS)
    assert counts.min() >= 1, "empty cluster unsupported"
    order = np.argsort(-counts, kind="stable")  # cluster ids, size desc

    n_ranks = N_CLUSTERS // N_CORES
    cids = np.empty((N_CORES, n_ranks), dtype=np.int64)
    for i, cid in enumerate(order):
        rnd, pos = divmod(i, N_CORES)
        core = pos if rnd % 2 == 0 else N_CORES - 1 - pos
        cids[core, rnd] = cid

    sizes = counts[cids]                      # [cores, ranks]
    Lmax = sizes.max(axis=0)                  # canonical per-rank size, desc

    # uniform-width bins: consecutive ranks (size desc) packed into 1024-col
    # bins; every cluster in a bin is padded to W = pad4(first rank's size)
    bins = []          # (r0, n, W)
    r0 = 0
    while r0 < n_ranks:
        W = int((Lmax[r0] + 3) // 4 * 4)
        n = min(BIN // W, n_ranks - r0, NK_MAX)
        bins.append((r0, int(n), W))
        r0 += n
    if len(bins) % 2:
        bins.append((n_ranks, 0, 0))          # empty bin pads to whole st

    L = np.zeros(n_ranks, dtype=np.int64)
    col0 = np.zeros(n_ranks, dtype=np.int64)
    for b, (r0, n, W) in enumerate(bins):
        for j in range(n):
            L[r0 + j] = W
            col0[r0 + j] = BIN * b + j * W
    S = BIN * len(bins)
    nk_rows = 96 + max(n for (_, n, _) in bins)

    # distinct one-hot geometries (n, W): a master pattern per geometry
    # lives in SBUF; per-bin it is copied on-chip instead of DMA'd from HBM
    geoms = []
    geom_of = []
    for (r0, n, W) in bins:
        key = (n, W)
        if n > 0 and key not in geoms:
            geoms.append(key)
        geom_of.append(geoms.index(key) if n > 0 else -1)

    sts = [(bins[b], bins[b + 1]) for b in range(0, len(bins), 2)]
    return dict(cids=cids, L=L, col0=col0, S=S, bins=bins, sts=sts,
                n_ranks=n_ranks, nk_rows=nk_rows, geoms=geoms,
                geom_of=geom_of)


def _prep_core(k, plan, rel_points, features, sort_idx, bucket0):
    """Per-core input arrays (canonical layout, core-specific data)."""
    col0, S, L = plan["col0"], plan["S"], plan["L"]
    cids = plan["cids"][k]
    n_ranks = plan["n_ranks"]
    nk_rows = plan["nk_rows"]

    # gap columns (bin tails) keep index 0; they are never reduced
    slot = np.zeros(S, dtype=np.int64)
    for r in range(n_ranks):
        cid = cids[r]
        idx = sort_idx[bucket0[cid]: bucket0[cid + 1]]
        n = idx.shape[0]
        c0 = col0[r]
        slot[c0: c0 + n] = idx
        if L[r] > n:
            slot[c0 + n: c0 + L[r]] = idx[0]

    pts = rel_points[slot]          # [S, 3] f32
    feat = features[slot]           # [S, 64] f32

    # encT rows: 0:3 points, 3:67 features, 96:96+nk bin-local one-hot
    encT = np.zeros((nk_rows, S), dtype=NPBF16)
    encT[0:3] = pts.T.astype(NPBF16)
    encT[3:67] = feat.T.astype(NPBF16)
    for b, (r0, n, W) in enumerate(plan["bins"]):
        for j in range(n):
            c0 = BIN * b + j * W
            encT[96 + j, c0:c0 + W] = NPBF16(1.0)

    pts4 = (
        pts.astype(NPBF16)
        .reshape(S // 4, 4, 3)
        .transpose(1, 2, 0)
        .reshape(12, S // 4)
    )
    return {"encT": encT, "pts4": np.ascontiguousarray(pts4)}


def _blockdiag(w, times):
    fi, fo = w.shape
    out = np.zeros((fi * times, fo * times), dtype=w.dtype)
    for i in range(times):
        out[i * fi:(i + 1) * fi, i * fo:(i + 1) * fo] = w
    return out


def _prep_weights(inp):
    W1 = inp["W1"]
    mats = {
        "enc1_lhsT": _blockdiag(inp["enc_W1"], 4),
        "enc2_lhsT": _blockdiag(inp["enc_W2"], 2),
        "enc2_hi": _blockdiag(inp["enc_W2"], 2),
        "W1ab": W1[0:67],
        "W1cb1": np.vstack([W1[67:131], inp["b1"][None]]),
        "fcW2": inp["W2"], "G1": inp["G1"],
        "G2a": inp["G2"][:, 0:128], "G2b": inp["G2"][:, 128:256],
    }
    wb16 = np.zeros((128, 128 * len(_WB16)), dtype=NPBF16)
    for i, (name, k, cols, pbase) in enumerate(_WB16):
        wb16[pbase:pbase + k, 128 * i:128 * i + cols] = mats[name].astype(NPBF16)

    vecs = {
        "b_enc1_4": np.tile(inp["enc_b1"], 4), "b_enc2": inp["enc_b2"],
        "b2": inp["b2"], "gb1": inp["gb1"],
        "gb2a": inp["gb2"][0:128], "gb2b": inp["gb2"][128:256],
    }
    wf32 = np.zeros((128, len(_WF32)), dtype=np.float32)
    for i, name in enumerate(_WF32):
        v = vecs[name]
        wf32[0:v.shape[0], i] = v
    return {"wb16": wb16, "wf32": wf32}


# ---------------------------------------------------------------- program

def _build(plan):
    S = plan["S"]
    n_ranks = plan["n_ranks"]
    nk_rows = plan["nk_rows"]
    nc = bacc.Bacc(None, target_bir_lowering=False, debug=True)

    encT_d = nc.dram_tensor("encT", [nk_rows, S], BF16, kind="ExternalInput")
    pts4_d = nc.dram_tensor("pts4", [12, S // 4], BF16, kind="ExternalInput")
    wb16_d = nc.dram_tensor("wb16", [128, 128 * len(_WB16)], BF16,
                            kind="ExternalInput")
    wf32_d = nc.dram_tensor("wf32", [128, len(_WF32)], F32, kind="ExternalInput")
    out_d = nc.dram_tensor("out", [256, 512], BF16, kind="ExternalOutput")

    RELU = mybir.ActivationFunctionType.Relu
    ADD = mybir.AluOpType.add
    MAX = mybir.AluOpType.max
    AX = mybir.AxisListType.X
    AXY = mybir.AxisListType.XY

    sts = plan["sts"]
    n_st = len(sts)
    W1AB_COL = 128 * 3  # W1ab offset in the bf16 blob
    N_SLOT = 4

    with tile.TileContext(nc) as tc, ExitStack() as ctx:
        consts = ctx.enter_context(tc.tile_pool(name="consts", bufs=1))
        glob = ctx.enter_context(tc.tile_pool(name="glob", bufs=1))
        enc_p = ctx.enter_context(tc.tile_pool(name="enc_p", bufs=4))
        h1_p = ctx.enter_context(tc.tile_pool(name="h1_p", bufs=2))
        e1_p = ctx.enter_context(tc.tile_pool(name="e1_p", bufs=2))
        sm_p = ctx.enter_context(tc.tile_pool(name="sm_p", bufs=3))
        psE = ctx.enter_context(tc.tile_pool(name="psE", bufs=1, space="PSUM"))
        psL = ctx.enter_context(tc.tile_pool(name="psL", bufs=3, space="PSUM"))

        # startup order: enc-stage weights + points + biases first (enough
        # for the prologue's enc matmuls), then the bulk of the blob
        wb16_t = consts.tile([128, 128 * len(_WB16)], BF16, tag="wb16")
        wf32_t = consts.tile([128, len(_WF32)], F32, tag="wf32")
        nc.sync.dma_start(out=wb16_t[:, 0:384], in_=wb16_d[:, 0:384])

        w_sb = {}
        for i, (name, k, cols, pbase) in enumerate(_WB16):
            w_sb[name] = wb16_t[pbase:pbase + k, 128 * i:128 * i + cols]
        bias = {}
        for i, name in enumerate(_WF32):
            rows = {"b_enc2": 64}.get(name, 128)
            bias[name] = wf32_t[0:rows, i:i + 1]

        # resident quad-packed points
        pts4_sb = consts.tile([12, S // 4], BF16, tag="pts4_sb")
        nc.sync.dma_start(out=pts4_sb[:], in_=pts4_d[:])
        nc.sync.dma_start(out=wf32_t[:], in_=wf32_d[:])
        nc.sync.dma_start(out=wb16_t[:, 384:], in_=wb16_d[:, 384:])

        # per-bin L1 lhsT slots: rows 0:67 = static W1ab, rows 96:96+nk = M,
        # rows 67:96 zero (paired with stale encT rows)
        lhsT_slots = [consts.tile([128, 128], BF16, tag=f"lhsT{i}",
                                  name=f"lhsT{i}") for i in range(N_SLOT)]
        for sl in lhsT_slots:
            nc.vector.memset(sl[64:96, :], 0.0)
            nc.scalar.dma_start(out=sl[0:67, :],
                                in_=wb16_d[0:67, W1AB_COL:W1AB_COL + 128])

        # neighT: [65, n_ranks]; row 64 = ones (folds b1 into the M matmul)
        neighT = glob.tile([65, n_ranks], BF16, tag="neighT")
        nc.vector.memset(neighT[64:65, :], 1.0)
        # stage-2 partials for the whole core
        T2 = glob.tile([128, n_ranks], BF16, tag="T2")

        # --------------- pipeline stages (per supertile) ---------------

        def enc_head(si):
            """enc1 matmul + relu evac; prefetch encT; returns psum+tiles."""
            (b0, b1) = sts[si]
            c0 = si * ST
            q0 = c0 // 4   # 512 quads per supertile
            ab = psE.tile([128, BIN], F32, tag="psE", name="ab")
            nc.tensor.matmul(ab[:, 0:512], w_sb["enc1_lhsT"],
                             pts4_sb[:, q0:q0 + 512], start=True, stop=True)
            h1 = h1_p.tile([128, 512], BF16, tag="h1")
            nc.scalar.activation(h1[:], ab[:, 0:512], RELU,
                                 bias=bias["b_enc1_4"], scale=1.0)
            # prefetch this supertile's encT (payload rows via SP DGE, the
            # small one-hot rows via the idle gpsimd DGE)
            encT_t = enc_p.tile([nk_rows, ST], BF16, tag="encT_t")
            if si < 4:
                # one-time per pool buffer: rows 67:96 stay zero forever
                # (uninitialized SBUF could hold NaN; 0 * NaN = NaN)
                nc.vector.memset(encT_t[64:96, :], 0.0)
            nc.sync.dma_start(out=encT_t[0:34, :],
                              in_=encT_d[0:34, c0:c0 + ST])
            nc.sync.dma_start(out=encT_t[34:67, :],
                              in_=encT_d[34:67, c0:c0 + ST])
            rows = 96 + max(b0[1], b1[1])
            nc.sync.dma_start(out=encT_t[96:rows, :],
                              in_=encT_d[96:rows, c0:c0 + ST])
            return ab, h1, encT_t

        def enc_tail(si, ab, h1):
            """enc2 matmuls + per-bin stage-1 reduce -> neighT columns."""
            (b0, b1) = sts[si]
            nk = b0[1] + b1[1]
            nc.tensor.matmul(ab[:, 0:512], w_sb["enc2_lhsT"], h1[0:64, :],
                             start=True, stop=True)
            nc.tensor.matmul(ab[:, 512:1024], w_sb["enc2_hi"], h1[64:128, :],
                             start=True, stop=True)
            # per-bin fused reduce over (half h, quad cols), separately for
            # the low/high 64-partition point groups (aligned starts), then
            # a partition-free tensor_max folds them -- no shift DMA needed
            mx = sm_p.tile([128, NK_MAX * 2], BF16, tag="mx")
            halves = ab[:].rearrange("p (h c) -> p h c", h=2)
            if b0[1] == b1[1] and b0[2] == b1[2] and b0[1] > 0:
                # both bins share (n, W): single 4-free-dim fused reduce
                n, W = b0[1], b0[2]
                wq = W // 4
                src = ab[:].rearrange("p (h b c) -> p b h c", h=2, b=2)
                src = src[:, :, :, 0:n * wq].rearrange(
                    "p b h (n w) -> p b n h w", w=wq)
                nc.vector.reduce_max(mx[:, 0:2 * n], src, axis=AXY)
            else:
                o = 0
                for t, (r0, n, W) in enumerate((b0, b1)):
                    if n == 0:
                        continue
                    wq = W // 4
                    q0 = t * (BIN // 4)
                    nc.vector.reduce_max(
                        mx[:, o:o + n],
                        halves[:, :, q0:q0 + n * wq]
                        .rearrange("p h (n w) -> p n h w", w=wq),
                        axis=AXY)
                    o += n
            # partition fold via the idle gpsimd iDMA (tiny copy); the
            # dependent mx2/bias ops are issued later (enc_tail_b) so the
            # in-order DVE queue never stalls waiting for this DMA
            fold = sm_p.tile([64, NK_MAX * 2], BF16, tag="fold")
            nc.gpsimd.dma_start(out=fold[:, :nk], in_=mx[64:128, :nk])
            return mx, fold

        def enc_tail_b(si, mx, fold):
            (b0, b1) = sts[si]
            nk = b0[1] + b1[1]
            mx2 = sm_p.tile([64, NK_MAX * 2], BF16, tag="mx2")
            nc.vector.tensor_max(mx2[:, :nk], mx[0:64, :nk], fold[:, :nk])
            r0 = b0[0]
            nc.vector.tensor_scalar(neighT[0:64, r0:r0 + nk], mx2[:, :nk],
                                    bias["b_enc2"], 0.0, op0=ADD, op1=MAX)

        def m_chain(si):
            """Per-bin M = [neigh;1]^T @ [W1c;b1] -> lhsT slot rows 67:67+nk."""
            (b0, b1) = sts[si]
            for t, (r0, n, W) in enumerate((b0, b1)):
                if n == 0:
                    continue
                pm = psL.tile([128, BIN], F32, tag="psL", name="pm")
                nc.tensor.matmul(pm[:n, 0:128], neighT[0:65, r0:r0 + n],
                                 w_sb["W1cb1"], start=True, stop=True)
                sl = lhsT_slots[(2 * si + t) % N_SLOT]
                nc.scalar.copy(sl[96:96 + n, :], pm[:n, 0:128])

        def l1_part(si, encT_t):
            """Per-bin L1 matmuls; relu evacs trail behind."""
            (b0, b1) = sts[si]
            e1 = e1_p.tile([128, ST], BF16, tag="e1")
            p1s = []
            for t, (r0, n, W) in enumerate((b0, b1)):
                fill = n * W
                sl = lhsT_slots[(2 * si + t) % N_SLOT]
                p1 = psL.tile([128, BIN], F32, tag="psL", name="p1")
                for a in (0, 512):
                    z = min(512, fill - a)
                    if z > 0:
                        nc.tensor.matmul(
                            p1[:, a:a + z], sl[0:96 + n, :],
                            encT_t[0:96 + n, t * BIN + a:t * BIN + a + z],
                            start=True, stop=True)
                p1s.append((p1, fill))
            for t, (p1, fill) in enumerate(p1s):
                if fill > 0:
                    nc.scalar.activation(e1[:, t * BIN:t * BIN + fill],
                                         p1[:, :fill], RELU)
            return e1

        def l2_part(si, e1):
            (b0, b1) = sts[si]
            for t, (r0, n, W) in enumerate((b0, b1)):
                fill = n * W
                if fill == 0:
                    continue
                p2 = psL.tile([128, BIN], F32, tag="psL", name="p2")
                for a in (0, 512):
                    z = min(512, fill - a)
                    if z > 0:
                        nc.tensor.matmul(
                            p2[:, a:a + z], w_sb["fcW2"],
                            e1[:, t * BIN + a:t * BIN + a + z],
                            start=True, stop=True)
                nc.vector.reduce_max(
                    T2[:, r0:r0 + n],
                    p2[:, 0:fill].rearrange("p (n w) -> p n w", w=W),
                    axis=AX)

        # --------------- software pipeline, lookahead 2 ---------------
        # per-iteration order keeps every tensor matmul's producer one full
        # iteration ahead: L1(si) needs M(si) (copied in iter si-1), the m
        # matmuls need neighT(si+1) (stage1 of iter si-1), L2 needs this
        # iteration's early Act evacs.
        # ---------------- global MLP tail (per rank chunk) ----------------
        gT = glob.tile([128, n_ranks], BF16, tag="gT")

        def tail_chunk(ck, r0, r1):
            z = r1 - r0
            if z <= 0:
                return
            nc.vector.tensor_scalar(gT[:, r0:r1], T2[:, r0:r1], bias["b2"],
                                    0.0, op0=ADD, op1=MAX)
            pg = psL.tile([128, BIN], F32, tag="psL", name="pg")
            nc.tensor.matmul(pg[:, 0:z], w_sb["G1"], gT[:, r0:r1],
                             start=True, stop=True)
            g1T = glob.tile([128, 512], BF16, tag=f"g1T{ck}")
            nc.scalar.activation(g1T[:, 0:z], pg[:, 0:z], RELU,
                                 bias=bias["gb1"], scale=1.0)
            for half, (wn, bn) in enumerate((("G2a", "gb2a"), ("G2b", "gb2b"))):
                po = psL.tile([128, BIN], F32, tag="psL", name="po")
                nc.tensor.matmul(po[:, 0:z], w_sb[wn], g1T[:, 0:z],
                                 start=True, stop=True)
                o_sb = glob.tile([128, 512], BF16, tag=f"osb{ck}_{half}")
                nc.scalar.activation(o_sb[:, 0:z], po[:, 0:z], RELU,
                                     bias=bias[bn], scale=1.0)
                nc.scalar.dma_start(
                    out=out_d[half * 128:(half + 1) * 128, r0:r1],
                    in_=o_sb[:, 0:z])

        # first tail chunk fires once its ranks' stage-2 maxes are final
        ranks_done = [sts[k][1][0] + sts[k][1][1] if sts[k][1][1] > 0
                      else sts[k][0][0] + sts[k][0][1] for k in range(n_st)]
        si_split = next((k for k, rd in enumerate(ranks_done)
                         if rd >= n_ranks // 2), n_st - 1)
        split_rank = ranks_done[si_split]

        # enc runs 3 supertiles ahead so the tiny partition-fold DMA gets a
        # full iteration of slack before mx2/neighT consume it on the DVE
        LA = 3
        pend = {}    # si -> (ab, h1, encT_t)
        mxs = {}     # si -> (mx, fold)
        for si in range(min(LA, n_st)):
            pend[si] = enc_head(si)
            mxs[si] = enc_tail(si, pend[si][0], pend[si][1])
        for si in range(min(LA - 1, n_st)):
            enc_tail_b(si, *mxs.pop(si))
        if n_st > 0:
            m_chain(0)
        for si in range(n_st):
            if si + LA < n_st:
                pend[si + LA] = enc_head(si + LA)
            e1 = l1_part(si, pend[si][2])
            if si + LA < n_st:
                mxs[si + LA] = enc_tail(si + LA, pend[si + LA][0],
                                        pend[si + LA][1])
            if si + 1 < n_st:
                m_chain(si + 1)
            l2_part(si, e1)
            if TAIL_SPLIT and si == si_split:
                tail_chunk(0, 0, split_rank)
            if si + LA - 1 in mxs:
                enc_tail_b(si + LA - 1, *mxs.pop(si + LA - 1))
            pend.pop(si)

        if TAIL_SPLIT:
            tail_chunk(1, split_rank, n_ranks)
        else:
            tail_chunk(1, 0, n_ranks)

    nc.finalize()
    return nc


# ---------------------------------------------------------------- entry

_CACHE = {}


def _run(inputs, trace=False, **spmd_kwargs):
    cluster = np.asarray(inputs["cluster"])
    key = hash(cluster.tobytes())
    if key not in _CACHE:
        plan = _plan(cluster)
        nc = _build(plan)
        _CACHE[key] = (plan, nc)
    plan, nc = _CACHE[key]

    rel_points = np.asarray(inputs["relative_points"], dtype=np.float32)
    features = np.asarray(inputs["features"], dtype=np.float32)
    sort_idx = np.argsort(cluster, kind="stable")
    bucket0 = np.concatenate(
        [[0], np.cumsum(np.bincount(cluster, minlength=N_CLUSTERS))]
    )
    wmap = _prep_weights({k: np.asarray(v, dtype=np.float32)
                          for k, v in inputs.items()
                          if k not in ("relative_points", "features", "cluster")})

    in_maps = []
    for k in range(N_CORES):
        m = _prep_core(k, plan, rel_points, features, sort_idx, bucket0)
        m.update(wmap)
        in_maps.append(m)

    res = run_bass_kernel_spmd(nc, in_maps, list(range(N_CORES)),
                               trace=trace, **spmd_kwargs)

    out = np.empty((N_CLUSTERS, 256), dtype=np.float32)
    for k in range(N_CORES):
        out[plan["cids"][k]] = res.results[k]["out"].T.astype(np.float32)
    return out, res


def kernel(**inputs):
    return _run(inputs)[0]


# revision 87
# speedup vs baseline: 1.0499x; 1.0085x over previous
"""Trainium2 Bass kernel v4 for nn_MiddleLayerEncoder (gnn_message_passing).

Strategy: shard by CLUSTER across 8 cores (512 whole clusters each, no
collectives).  Host prep sorts points by cluster and packs whole clusters
into 1024-column bins with a UNIFORM padded width per bin (canonical
across cores -> single SPMD program).  Uniform widths make every segment
reduce a single strided DVE op per bin.

Per-bin L1 lhsT slots: rows 0:67 = static W1ab (preloaded once), rows
96:96+nk = per-bin M (the per-cluster term M = [neigh;1]^T @ [W1c;b1],
evacuated from PSUM by an Act copy; 96 keeps the copy quadrant-aligned,
rows 67:96 are zeroed once so stale encT rows multiply to zero).  encT
carries payload rows 0:67 (pts 3 + feat 64) and nk bin-local one-hot
rows at 96, so the L1 matmul injects M per point for free in K.

Segment maxes are one DVE reduce per bin directly from PSUM (stage1: 4D
[p,n,h,wq] fused over the two enc2 halves; stage2: 3D [p,n,W]); the tiny
partition fold rides the idle gpsimd iDMA with a full iteration of slack
(LA=3 enc lookahead) so the in-order DVE queue never waits on it.  PSUM:
psE(1x2banks) enc, psL(3x2banks) ring = L1 b0/b1, next st's two M
matmuls, L2 b0/b1.  The encT stream is split into three SP-ring DMAs per
supertile -- empirically the sweet spot for the greedy descriptor
dispatcher that otherwise piles everything onto one DMA queue.  Output
is written bf16 (widened on host).
"""

import numpy as np
import ml_dtypes
from contextlib import ExitStack

import concourse.bass as bass
import concourse.bacc as bacc
import concourse.tile as tile
from concourse import mybir
from concourse.bass_utils import run_bass_kernel_spmd

BF16 = mybir.dt.bfloat16
F32 = mybir.dt.float32
NPBF16 = ml_dtypes.bfloat16

N_CORES = 8
N_PTS = 262144
TAIL_SPLIT = False
N_CLUSTERS = 4096
BIN = 1024
ST = 2 * BIN
NK_MAX = 30        # one-hot rows per bin: 67 + nk <= 128 (and <= 97 here)

# bf16 weight blob layout: (name, K rows, cols, partition base)
_WB16 = [
    ("enc1_lhsT", 12, 128, 0), ("enc2_lhsT", 64, 128, 0),
    ("enc2_hi", 64, 128, 64), ("W1ab", 67, 128, 0), ("W1cb1", 65, 128, 0),
    ("fcW2", 128, 128, 0), ("G1", 128, 128, 0),
    ("G2a", 128, 128, 0), ("G2b", 128, 128, 0),
]
_WF32 = ["b_enc1_4", "b_enc2", "b2", "gb1", "gb2a", "gb2b"]  # one f32 col each


# ---------------------------------------------------------------- planning

def _plan(cluster):
    """Canonical SPMD layout shared by all cores (uniform width per bin)."""
    counts = np.bincount(cluster, minlength=N_CLUSTERS)
    assert counts.min() >= 1, "empty cluster unsupported"
    order = np.argsort(-counts, kind="stable")  # cluster ids, size desc

    n_ranks = N_CLUSTERS // N_CORES
    cids = np.empty((N_CORES, n_ranks), dtype=np.int64)
    for i, cid in enumerate(order):
        rnd, pos = divmod(i, N_CORES)
        core = pos if rnd % 2 == 0 else N_CORES - 1 - pos
        cids[core, rnd] = cid

    sizes = counts[cids]                      # [cores, ranks]
    Lmax = sizes.max(axis=0)                  # canonical per-rank size, desc

    # uniform-width bins: consecutive ranks (size desc) packed into 1024-col
    # bins; every cluster in a bin is padded to W = pad4(first rank's size)
    bins = []          # (r0, n, W)
    r0 = 0
    while r0 < n_ranks:
        W = int((Lmax[r0] + 3) // 4 * 4)
        n = min(BIN // W, n_ranks - r0, NK_MAX)
        bins.append((r0, int(n), W))
        r0 += n
    if len(bins) % 2:
        bins.append((n_ranks, 0, 0))          # empty bin pads to whole st

    L = np.zeros(n_ranks, dtype=np.int64)
    col0 = np.zeros(n_ranks, dtype=np.int64)
    for b, (r0, n, W) in enumerate(bins):
        for j in range(n):
            L[r0 + j] = W
            col0[r0 + j] = BIN * b + j * W
    S = BIN * len(bins)
    nk_rows = 96 + max(n for (_, n, _) in bins)

    # distinct one-hot geometries (n, W): a master pattern per geometry
    # lives in SBUF; per-bin it is copied on-chip instead of DMA'd from HBM
    geoms = []
    geom_of = []
    for (r0, n, W) in bins:
        key = (n, W)
        if n > 0 and key not in geoms:
            geoms.append(key)
        geom_of.append(geoms.index(key) if n > 0 else -1)

    sts = [(bins[b], bins[b + 1]) for b in range(0, len(bins), 2)]
    return dict(cids=cids, L=L, col0=col0, S=S, bins=bins, sts=sts,
                n_ranks=n_ranks, nk_rows=nk_rows, geoms=geoms,
                geom_of=geom_of)


def _prep_core(k, plan, rel_points, features, sort_idx, bucket0):
    """Per-core input arrays (canonical layout, core-specific data)."""
    col0, S, L = plan["col0"], plan["S"], plan["L"]
    cids = plan["cids"][k]
    n_ranks = plan["n_ranks"]
    nk_rows = plan["nk_rows"]

    # gap columns (bin tails) keep index 0; they are never reduced
    slot = np.zeros(S, dtype=np.int64)
    for r in range(n_ranks):
        cid = cids[r]
        idx = sort_idx[bucket0[cid]: bucket0[cid + 1]]
        n = idx.shape[0]
        c0 = col0[r]
        slot[c0: c0 + n] = idx
        if L[r] > n:
            slot[c0 + n: c0 + L[r]] = idx[0]

    pts = rel_points[slot]          # [S, 3] f32
    feat = features[slot]           # [S, 64] f32

    # encT rows: 0:3 points, 3:67 features, 96:96+nk bin-local one-hot
    encT = np.zeros((nk_rows, S), dtype=NPBF16)
    encT[0:3] = pts.T.astype(NPBF16)
    encT[3:67] = feat.T.astype(NPBF16)
    for b, (r0, n, W) in enumerate(plan["bins"]):
        for j in range(n):
            c0 = BIN * b + j * W
            encT[96 + j, c0:c0 + W] = NPBF16(1.0)

    pts4 = (
        pts.astype(NPBF16)
        .reshape(S // 4, 4, 3)
        .transpose(1, 2, 0)
        .reshape(12, S // 4)
    )
    return {"encT": encT, "pts4": np.ascontiguousarray(pts4)}


def _blockdiag(w, times):
    fi, fo = w.shape
    out = np.zeros((fi * times, fo * times), dtype=w.dtype)
    for i in range(times):
        out[i * fi:(i + 1) * fi, i * fo:(i + 1) * fo] = w
    return out


def _prep_weights(inp):
    W1 = inp["W1"]
    mats = {
        "enc1_lhsT": _blockdiag(inp["enc_W1"], 4),
        "enc2_lhsT": _blockdiag(inp["enc_W2"], 2),
        "enc2_hi": _blockdiag(inp["enc_W2"], 2),
        "W1ab": W1[0:67],
        "W1cb1": np.vstack([W1[67:131], inp["b1"][None]]),
        "fcW2": inp["W2"], "G1": inp["G1"],
        "G2a": inp["G2"][:, 0:128], "G2b": inp["G2"][:, 128:256],
    }
    wb16 = np.zeros((128, 128 * len(_WB16)), dtype=NPBF16)
    for i, (name, k, cols, pbase) in enumerate(_WB16):
        wb16[pbase:pbase + k, 128 * i:128 * i + cols] = mats[name].astype(NPBF16)

    vecs = {
        "b_enc1_4": np.tile(inp["enc_b1"], 4), "b_enc2": inp["enc_b2"],
        "b2": inp["b2"], "gb1": inp["gb1"],
        "gb2a": inp["gb2"][0:128], "gb2b": inp["gb2"][128:256],
    }
    wf32 = np.zeros((128, len(_WF32)), dtype=np.float32)
    for i, name in enumerate(_WF32):
        v = vecs[name]
        wf32[0:v.shape[0], i] = v
    return {"wb16": wb16, "wf32": wf32}


# ---------------------------------------------------------------- program

def _build(plan):
    S = plan["S"]
    n_ranks = plan["n_ranks"]
    nk_rows = plan["nk_rows"]
    nc = bacc.Bacc(None, target_bir_lowering=False, debug=True)

    encT_d = nc.dram_tensor("encT", [nk_rows, S], BF16, kind="ExternalInput")
    pts4_d = nc.dram_tensor("pts4", [12, S // 4], BF16, kind="ExternalInput")
    wb16_d = nc.dram_tensor("wb16", [128, 128 * len(_WB16)], BF16,
                            kind="ExternalInput")
    wf32_d = nc.dram_tensor("wf32", [128, len(_WF32)], F32, kind="ExternalInput")
    out_d = nc.dram_tensor("out", [256, 512], BF16, kind="ExternalOutput")

    RELU = mybir.ActivationFunctionType.Relu
    ADD = mybir.AluOpType.add
    MAX = mybir.AluOpType.max
    AX = mybir.AxisListType.X
    AXY = mybir.AxisListType.XY

    sts = plan["sts"]
    n_st = len(sts)
    W1AB_COL = 128 * 3  # W1ab offset in the bf16 blob
    N_SLOT = 4

    with tile.TileContext(nc) as tc, ExitStack() as ctx:
        consts = ctx.enter_context(tc.tile_pool(name="consts", bufs=1))
        glob = ctx.enter_context(tc.tile_pool(name="glob", bufs=1))
        enc_p = ctx.enter_context(tc.tile_pool(name="enc_p", bufs=4))
        h1_p = ctx.enter_context(tc.tile_pool(name="h1_p", bufs=2))
        e1_p = ctx.enter_context(tc.tile_pool(name="e1_p", bufs=2))
        sm_p = ctx.enter_context(tc.tile_pool(name="sm_p", bufs=3))
        psE = ctx.enter_context(tc.tile_pool(name="psE", bufs=1, space="PSUM"))
        psL = ctx.enter_context(tc.tile_pool(name="psL", bufs=3, space="PSUM"))

        # startup order: enc-stage weights + points + biases first (enough
        # for the prologue's enc matmuls), then the bulk of the blob
        wb16_t = consts.tile([128, 128 * len(_WB16)], BF16, tag="wb16")
        wf32_t = consts.tile([128, len(_WF32)], F32, tag="wf32")
        nc.sync.dma_start(out=wb16_t[:, 0:384], in_=wb16_d[:, 0:384])

        w_sb = {}
        for i, (name, k, cols, pbase) in enumerate(_WB16):
            w_sb[name] = wb16_t[pbase:pbase + k, 128 * i:128 * i + cols]
        bias = {}
        for i, name in enumerate(_WF32):
            rows = {"b_enc2": 64}.get(name, 128)
            bias[name] = wf32_t[0:rows, i:i + 1]

        # resident quad-packed points
        pts4_sb = consts.tile([12, S // 4], BF16, tag="pts4_sb")
        nc.sync.dma_start(out=pts4_sb[:], in_=pts4_d[:])
        nc.sync.dma_start(out=wf32_t[:], in_=wf32_d[:])
        nc.sync.dma_start(out=wb16_t[:, 384:], in_=wb16_d[:, 384:])

        # per-bin L1 lhsT slots: rows 0:67 = static W1ab, rows 96:96+nk = M,
        # rows 67:96 zero (paired with stale encT rows)
        lhsT_slots = [consts.tile([128, 128], BF16, tag=f"lhsT{i}",
                                  name=f"lhsT{i}") for i in range(N_SLOT)]
        for sl in lhsT_slots:
            nc.vector.memset(sl[64:96, :], 0.0)
            nc.scalar.dma_start(out=sl[0:67, :],
                                in_=wb16_d[0:67, W1AB_COL:W1AB_COL + 128])

        # neighT: [65, n_ranks]; row 64 = ones (folds b1 into the M matmul)
        neighT = glob.tile([65, n_ranks], BF16, tag="neighT")
        nc.vector.memset(neighT[64:65, :], 1.0)
        # stage-2 partials for the whole core
        T2 = glob.tile([128, n_ranks], BF16, tag="T2")

        # --------------- pipeline stages (per supertile) ---------------

        def enc_head(si):
            """enc1 matmul + relu evac; prefetch encT; returns psum+tiles."""
            (b0, b1) = sts[si]
            c0 = si * ST
            q0 = c0 // 4   # 512 quads per supertile
            ab = psE.tile([128, BIN], F32, tag="psE", name="ab")
            nc.tensor.matmul(ab[:, 0:512], w_sb["enc1_lhsT"],
                             pts4_sb[:, q0:q0 + 512], start=True, stop=True)
            h1 = h1_p.tile([128, 512], BF16, tag="h1")
            nc.scalar.activation(h1[:], ab[:, 0:512], RELU,
                                 bias=bias["b_enc1_4"], scale=1.0)
            # prefetch this supertile's encT (payload rows via SP DGE, the
            # small one-hot rows via the idle gpsimd DGE)
            encT_t = enc_p.tile([nk_rows, ST], BF16, tag="encT_t")
            if si < 4:
                # one-time per pool buffer: rows 67:96 stay zero forever
                # (uninitialized SBUF could hold NaN; 0 * NaN = NaN)
                nc.vector.memset(encT_t[64:96, :], 0.0)
            nc.sync.dma_start(out=encT_t[0:34, :],
                              in_=encT_d[0:34, c0:c0 + ST])
            nc.sync.dma_start(out=encT_t[34:67, :],
                              in_=encT_d[34:67, c0:c0 + ST])
            rows = 96 + max(b0[1], b1[1])
            nc.sync.dma_start(out=encT_t[96:rows, :],
                              in_=encT_d[96:rows, c0:c0 + ST])
            return ab, h1, encT_t

        def enc_tail(si, ab, h1):
            """enc2 matmuls + per-bin stage-1 reduce -> neighT columns."""
            (b0, b1) = sts[si]
            nk = b0[1] + b1[1]
            nc.tensor.matmul(ab[:, 0:512], w_sb["enc2_lhsT"], h1[0:64, :],
                             start=True, stop=True)
            nc.tensor.matmul(ab[:, 512:1024], w_sb["enc2_hi"], h1[64:128, :],
                             start=True, stop=True)
            # per-bin fused reduce over (half h, quad cols), separately for
            # the low/high 64-partition point groups (aligned starts), then
            # a partition-free tensor_max folds them -- no shift DMA needed
            mx = sm_p.tile([128, NK_MAX * 2], BF16, tag="mx")
            halves = ab[:].rearrange("p (h c) -> p h c", h=2)
            o = 0
            for t, (r0, n, W) in enumerate((b0, b1)):
                if n == 0:
                    continue
                wq = W // 4
                q0 = t * (BIN // 4)
                nc.vector.reduce_max(
                    mx[:, o:o + n],
                    halves[:, :, q0:q0 + n * wq]
                    .rearrange("p h (n w) -> p n h w", w=wq),
                    axis=AXY)
                o += n
            # partition fold via the idle gpsimd iDMA (tiny copy); the
            # dependent mx2/bias ops are issued later (enc_tail_b) so the
            # in-order DVE queue never stalls waiting for this DMA
            fold = sm_p.tile([64, NK_MAX * 2], BF16, tag="fold")
            nc.gpsimd.dma_start(out=fold[:, :nk], in_=mx[64:128, :nk])
            return mx, fold

        def enc_tail_b(si, mx, fold):
            (b0, b1) = sts[si]
            nk = b0[1] + b1[1]
            mx2 = sm_p.tile([64, NK_MAX * 2], BF16, tag="mx2")
            nc.vector.tensor_max(mx2[:, :nk], mx[0:64, :nk], fold[:, :nk])
            r0 = b0[0]
            nc.vector.tensor_scalar(neighT[0:64, r0:r0 + nk], mx2[:, :nk],
                                    bias["b_enc2"], 0.0, op0=ADD, op1=MAX)

        def m_chain(si):
            """Per-bin M = [neigh;1]^T @ [W1c;b1] -> lhsT slot rows 67:67+nk."""
            (b0, b1) = sts[si]
            for t, (r0, n, W) in enumerate((b0, b1)):
                if n == 0:
                    continue
                pm = psL.tile([128, BIN], F32, tag="psL", name="pm")
                nc.tensor.matmul(pm[:n, 0:128], neighT[0:65, r0:r0 + n],
                                 w_sb["W1cb1"], start=True, stop=True)
                sl = lhsT_slots[(2 * si + t) % N_SLOT]
                nc.scalar.copy(sl[96:96 + n, :], pm[:n, 0:128])

        def l1_part(si, encT_t):
            """Per-bin L1 matmuls; relu evacs trail behind."""
            (b0, b1) = sts[si]
            e1 = e1_p.tile([128, ST], BF16, tag="e1")
            p1s = []
            for t, (r0, n, W) in enumerate((b0, b1)):
                fill = n * W
                sl = lhsT_slots[(2 * si + t) % N_SLOT]
                p1 = psL.tile([128, BIN], F32, tag="psL", name="p1")
                for a in (0, 512):
                    z = min(512, fill - a)
                    if z > 0:
                        nc.tensor.matmul(
                            p1[:, a:a + z], sl[0:96 + n, :],
                            encT_t[0:96 + n, t * BIN + a:t * BIN + a + z],
                            start=True, stop=True)
                p1s.append((p1, fill))
            for t, (p1, fill) in enumerate(p1s):
                if fill > 0:
                    nc.scalar.activation(e1[:, t * BIN:t * BIN + fill],
                                         p1[:, :fill], RELU)
            return e1

        def l2_part(si, e1):
            (b0, b1) = sts[si]
            for t, (r0, n, W) in enumerate((b0, b1)):
                fill = n * W
                if fill == 0:
                    continue
                p2 = psL.tile([128, BIN], F32, tag="psL", name="p2")
                for a in (0, 512):
                    z = min(512, fill - a)
                    if z > 0:
                        nc.tensor.matmul(
                            p2[:, a:a + z], w_sb["fcW2"],
                            e1[:, t * BIN + a:t * BIN + a + z],
                            start=True, stop=True)
                nc.vector.reduce_max(
                    T2[:, r0:r0 + n],
                    p2[:, 0:fill].rearrange("p (n w) -> p n w", w=W),
                    axis=AX)

        # --------------- software pipeline, lookahead 2 ---------------
        # per-iteration order keeps every tensor matmul's producer one full
        # iteration ahead: L1(si) needs M(si) (copied in iter si-1), the m
        # matmuls need neighT(si+1) (stage1 of iter si-1), L2 needs this
        # iteration's early Act evacs.
        # ---------------- global MLP tail (per rank chunk) ----------------
        gT = glob.tile([128, n_ranks], BF16, tag="gT")

        def tail_chunk(ck, r0, r1):
            z = r1 - r0
            if z <= 0:
                return
            nc.vector.tensor_scalar(gT[:, r0:r1], T2[:, r0:r1], bias["b2"],
                                    0.0, op0=ADD, op1=MAX)
            pg = psL.tile([128, BIN], F32, tag="psL", name="pg")
            nc.tensor.matmul(pg[:, 0:z], w_sb["G1"], gT[:, r0:r1],
                             start=True, stop=True)
            g1T = glob.tile([128, 512], BF16, tag=f"g1T{ck}")
            nc.scalar.activation(g1T[:, 0:z], pg[:, 0:z], RELU,
                                 bias=bias["gb1"], scale=1.0)
            for half, (wn, bn) in enumerate((("G2a", "gb2a"), ("G2b", "gb2b"))):
                po = psL.tile([128, BIN], F32, tag="psL", name="po")
                nc.tensor.matmul(po[:, 0:z], w_sb[wn], g1T[:, 0:z],
                                 start=True, stop=True)
                o_sb = glob.tile([128, 512], BF16, tag=f"osb{ck}_{half}")
                nc.scalar.activation(o_sb[:, 0:z], po[:, 0:z], RELU,
                                     bias=bias[bn], scale=1.0)
                nc.scalar.dma_start(
                    out=out_d[half * 128:(half + 1) * 128, r0:r1],
                    in_=o_sb[:, 0:z])

        # first tail chunk fires once its ranks' stage-2 maxes are final
        ranks_done = [sts[k][1][0] + sts[k][1][1] if sts[k][1][1] > 0
                      else sts[k][0][0] + sts[k][0][1] for k in range(n_st)]
        si_split = next((k for k, rd in enumerate(ranks_done)
                         if rd >= n_ranks // 2), n_st - 1)
        split_rank = ranks_done[si_split]

        # enc runs 3 supertiles ahead so the tiny partition-fold DMA gets a
        # full iteration of slack before mx2/neighT consume it on the DVE
        LA = 3
        pend = {}    # si -> (ab, h1, encT_t)
        mxs = {}     # si -> (mx, fold)
        for si in range(min(LA, n_st)):
            pend[si] = enc_head(si)
            mxs[si] = enc_tail(si, pend[si][0], pend[si][1])
        for si in range(min(LA - 1, n_st)):
            enc_tail_b(si, *mxs.pop(si))
        if n_st > 0:
            m_chain(0)
        for si in range(n_st):
            if si + LA < n_st:
                pend[si + LA] = enc_head(si + LA)
            e1 = l1_part(si, pend[si][2])
            if si + LA < n_st:
                mxs[si + LA] = enc_tail(si + LA, pend[si + LA][0],
                                        pend[si + LA][1])
            if si + 1 < n_st:
                m_chain(si + 1)
            l2_part(si, e1)
            if TAIL_SPLIT and si == si_split:
                tail_chunk(0, 0, split_rank)
            if si + LA - 1 in mxs:
                enc_tail_b(si + LA - 1, *mxs.pop(si + LA - 1))
            pend.pop(si)

        if TAIL_SPLIT:
            tail_chunk(1, split_rank, n_ranks)
        else:
            tail_chunk(1, 0, n_ranks)

    nc.finalize()
    return nc


# ---------------------------------------------------------------- entry

_CACHE = {}


def _run(inputs, trace=False, **spmd_kwargs):
    cluster = np.asarray(inputs["cluster"])
    key = hash(cluster.tobytes())
    if key not in _CACHE:
        plan = _plan(cluster)
        nc = _build(plan)
        _CACHE[key] = (plan, nc)
    plan, nc = _CACHE[key]

    rel_points = np.asarray(inputs["relative_points"], dtype=np.float32)
    features = np.asarray(inputs["features"], dtype=np.float32)
    sort_idx = np.argsort(cluster, kind="stable")
    bucket0 = np.concatenate(
        [[0], np.cumsum(np.bincount(cluster, minlength=N_CLUSTERS))]
    )
    wmap = _prep_weights({k: np.asarray(v, dtype=np.float32)
                          for k, v in inputs.items()
                          if k not in ("relative_points", "features", "cluster")})

    in_maps = []
    for k in range(N_CORES):
        m = _prep_core(k, plan, rel_points, features, sort_idx, bucket0)
        m.update(wmap)
        in_maps.append(m)

    res = run_bass_kernel_spmd(nc, in_maps, list(range(N_CORES)),
                               trace=trace, **spmd_kwargs)

    out = np.empty((N_CLUSTERS, 256), dtype=np.float32)
    for k in range(N_CORES):
        out[plan["cids"][k]] = res.results[k]["out"].T.astype(np.float32)
    return out, res


def kernel(**inputs):
    return _run(inputs)[0]
